# revision 17
# baseline (speedup 1.0000x reference)
"""Trainium2 Bass kernel for a fused MultiHeadAttention block.

Reference computation (B=4, S=1024, D=1024, H=16, DK=DV=64):
    qh = einsum('bqd,hdk->bhqk', q, wq); kh, vh likewise
    attn = softmax(mask_fill(qh/sqrt(DK) @ kh^T))
    out  = LayerNorm(concat_heads(attn @ vh) @ fc_w.T + q) * ln_g + ln_b

Sharding: 8 shards = (batch b, seq half).  Each core owns 512 query rows of
one batch; K/V projections for that batch are computed redundantly by the
core pair.  Zero collectives.

Per-core strategy (bf16 everywhere, zero PE transposes, minimal PE
moving-columns — the HW is bound by PE column streaming at ~0.8ns/col):
  - all inputs/weights are cast to bf16 on the host; weights are also
    pre-arranged into their SBUF layouts (partition-major, >=4KB DMA rows)
    and wq is pre-scaled by 1/temperature; mask is pre-transposed to
    [k, q] bf16.
  - q/k/v are transposed on the way in by the DMA engines' XBAR
    (dma_start_transpose, 2-byte dtypes) — the PE does only real matmuls.
  - scores are computed TRANSPOSED [k_part, q_free], so softmax needs no
    max pass (|scores| <~ 6 sigma, exp cannot overflow) and no transpose of
    the 8.4M-element score tensor.
  - masking is p = exp(scores) * mask  (bitwise-identical to the reference's
    -1e9 masked-fill + softmax: both give exactly 0 weight).
  - PV runs with p as the STATIONARY operand and vh as the moving one, so
    its output lands in [q_part, dv] orientation with only 65 moving
    columns per matmul (64 dv + a ones-column that yields the softmax
    row-sum for free) — half the columns of the [dv, q] orientation.
    Normalization is then a per-partition reciprocal+tensor_scalar (no
    partition broadcast needed).
  - per-pair head outputs bounce through DRAM and are XBAR-transposed back
    into the [hv_part, q] layout the fc matmul needs; both transfers
    overlap attention of later pairs.
  - rel err vs the fp32 reference: ~1.8e-3 (bf16-dominated).
"""

import os
import sys

import numpy as np

for _p in ("/opt/trn_rl_repo",):
    if _p not in sys.path and os.path.isdir(_p):
        sys.path.insert(0, _p)

from contextlib import ExitStack

import ml_dtypes

import concourse.bass as bass
import concourse.tile as tile
from concourse import bacc, mybir
from concourse.bass_utils import run_bass_kernel_spmd

F32 = mybir.dt.float32
BF16 = mybir.dt.bfloat16
AF = mybir.ActivationFunctionType

B, S, D = 4, 1024, 1024
H, DK, DV = 16, 64, 64
SQ = S // 2          # query rows per core
P = 128
NDC = D // P         # 8 contraction chunks over D
NKC = S // P         # 8 key chunks
NQT = SQ // P        # 4 query subtiles
NPAIR = H // 2       # 8 head pairs
LN_EPS = 1e-6
N_CORES = 8
BF = ml_dtypes.bfloat16


def build_program(reps: int = 1):
    nc = bacc.Bacc("TRN2", target_bir_lowering=False, debug=False)

    q_d = nc.dram_tensor("q_sh", [SQ, D], BF16, kind="ExternalInput")
    k_d = nc.dram_tensor("k_full", [S, D], BF16, kind="ExternalInput")
    v_d = nc.dram_tensor("v_full", [S, D], BF16, kind="ExternalInput")
    mT_d = nc.dram_tensor("maskT_sh", [S, SQ], BF16, kind="ExternalInput")
    wkq_d = nc.dram_tensor("wkq_l", [NPAIR, P, NDC, 4 * DK], BF16,
                           kind="ExternalInput")
    wv_d = nc.dram_tensor("wv_l", [P, NDC, H * DV], BF16, kind="ExternalInput")
    fc_d = nc.dram_tensor("fcT_l", [P, NDC, D], BF16, kind="ExternalInput")
    g_d = nc.dram_tensor("ln_g", [D], F32, kind="ExternalInput")
    b_d = nc.dram_tensor("ln_b", [D], F32, kind="ExternalInput")
    o_d = nc.dram_tensor("out_sh", [SQ, D], F32, kind="ExternalOutput")

    with tile.TileContext(nc) as tc, ExitStack() as ctx:
        singles = ctx.enter_context(tc.tile_pool(name="singles", bufs=1))
        bigs = ctx.enter_context(tc.tile_pool(name="bigs", bufs=1))
        work = ctx.enter_context(tc.tile_pool(name="work", bufs=2))
        pwork = ctx.enter_context(tc.tile_pool(name="pwork", bufs=8))

        zero1 = singles.tile([P, 1], F32, tag="zero1")
        nc.vector.memset(zero1, 0.0)
        eps1 = singles.tile([P, 1], F32, tag="eps1")
        nc.vector.memset(eps1, LN_EPS)

        def _one_rep():
            # --------------------------------------------------------------
            # Loads.  SP queue: v/k/q DMA-transposes + per-pair wkq.
            # Act queue: wv, mask, fc, residual, ln params.
            # --------------------------------------------------------------
            vT = bigs.tile([P, NDC, S], BF16, tag="vT")
            kT = bigs.tile([P, NDC, S], BF16, tag="kT")
            qT = bigs.tile([P, NDC, SQ], BF16, tag="qT")
            maskT = bigs.tile([P, NKC, SQ], BF16, tag="maskT")
            wv_sb = bigs.tile([P, NDC, H * DV], BF16, tag="wv")
            fcT = bigs.tile([P, NDC, D], BF16, tag="fcT")
            qres = bigs.tile([P, NQT, D], BF16, tag="qres")
            gb = bigs.tile([P, 2, D], F32, tag="gb")

            # SP queue carries everything the next rep's PE start needs, so
            # it prefetches during the previous rep's phase D (the out DMAs
            # live on the Act queue and don't block it).
            nc.sync.dma_start_transpose(
                vT[:, :, 0:P], v_d[0:P, :])
            nc.sync.dma_start(out=wv_sb[:, :, 0:512], in_=wv_d[:, :, 0:512])
            nc.sync.dma_start(out=wv_sb[:, :, 512:], in_=wv_d[:, :, 512:])
            for kc in range(1, NKC):
                nc.sync.dma_start_transpose(
                    vT[:, :, kc * P:(kc + 1) * P],
                    v_d[kc * P:(kc + 1) * P, :])
            for half in range(2):
                nc.sync.dma_start_transpose(
                    kT[:, :, half * SQ:(half + 1) * SQ],
                    k_d[half * SQ:(half + 1) * SQ, :])
            nc.sync.dma_start_transpose(qT, q_d.ap())
            nc.scalar.dma_start(
                out=maskT, in_=mT_d.ap().rearrange("(kc p) q -> p kc q", p=P))
            nc.scalar.dma_start(out=fcT, in_=fc_d.ap())
            nc.scalar.dma_start(
                out=qres, in_=q_d.ap().rearrange("(t p) d -> p t d", p=P))
            nc.scalar.dma_start(
                out=gb[:, 0, :], in_=g_d.ap().unsqueeze(0).to_broadcast([P, D]))
            nc.scalar.dma_start(
                out=gb[:, 1, :], in_=b_d.ap().unsqueeze(0).to_broadcast([P, D]))

            # --------------------------------------------------------------
            # Phase B: vh for all heads.  vh_sb[kp, kc, h, 0:64] = vh,
            # col 64 = 1, cols 65:128 = 0 (free softmax row-sums).
            # --------------------------------------------------------------
            vh_sb = bigs.tile([P, NKC, H, 2 * DV], BF16, tag="vh")
            nc.vector.memset(vh_sb[:, :, :, DV:].bitcast(mybir.dt.uint32), 0)
            nc.vector.memset(
                vh_sb[:, :, :, DV:DV + 2].bitcast(mybir.dt.uint32),
                0x00003F80)  # bf16 pair [1.0, 0.0] little-endian

            with tc.tile_pool(name="ps_b", bufs=4, space="PSUM") as ps_b:
                for kc in range(NKC):
                    for half in range(2):
                        vps = ps_b.tile([P, 512], F32, tag="vps")
                        for dj in range(NDC):
                            nc.tensor.matmul(
                                vps, lhsT=vT[:, dj, kc * P:(kc + 1) * P],
                                rhs=wv_sb[:, dj, half * 512:(half + 1) * 512],
                                start=(dj == 0), stop=(dj == NDC - 1))
                        nc.scalar.copy(
                            out=vh_sb[:, kc, half * 8:(half + 1) * 8, 0:DV],
                            in_=vps.rearrange("p (h v) -> p h v", v=DV))

            # --------------------------------------------------------------
            # Phase C: per head-pair projections + attention.
            # concatT[ip, pair, q] rows: head 2*pair in 0:64, 2*pair+1 in
            # 64:128
            # --------------------------------------------------------------
            concatT = bigs.tile([P, NPAIR, SQ], BF16, tag="concatT")

            with (
                tc.tile_pool(name="ps_kq", bufs=2, space="PSUM") as ps_kq,
                tc.tile_pool(name="ps_sc", bufs=2, space="PSUM") as ps_sc,
                tc.tile_pool(name="ps_hd", bufs=2, space="PSUM") as ps_hd,
                tc.tile_pool(name="cbounce", bufs=2, space="DRAM") as cbounce,
            ):
                for pair in range(NPAIR):
                    wkq = work.tile([P, NDC, 4 * DK], BF16, tag="wkq")
                    nc.sync.dma_start(out=wkq, in_=wkq_d[pair])

                    # khT2: [128 (2h x dk), 1024 kcol]
                    khT2 = work.tile([P, S], BF16, tag="khT2")
                    for half in range(2):
                        khps = ps_kq.tile([P, 512], F32, tag="kqps")
                        for dj in range(NDC):
                            nc.tensor.matmul(
                                khps, lhsT=wkq[:, dj, 0:2 * DK],
                                rhs=kT[:, dj, half * 512:(half + 1) * 512],
                                start=(dj == 0), stop=(dj == NDC - 1))
                        nc.vector.tensor_copy(
                            out=khT2[:, half * 512:(half + 1) * 512], in_=khps)
                    # qhT2: [128 (2h x dk), 512 q]  (wq pre-scaled by 1/temp)
                    qhT2 = work.tile([P, SQ], BF16, tag="qhT2")
                    qhps = ps_kq.tile([P, SQ], F32, tag="kqps")
                    for dj in range(NDC):
                        nc.tensor.matmul(
                            qhps, lhsT=wkq[:, dj, 2 * DK:4 * DK],
                            rhs=qT[:, dj, :],
                            start=(dj == 0), stop=(dj == NDC - 1))
                    nc.vector.tensor_copy(out=qhT2, in_=qhps)

                    # heads_sb[q_part, qt, hl*64+dv] -- [q, dv] orientation
                    heads_sb = work.tile([P, NQT, 2 * DV], BF16, tag="heads")
                    # All probabilities for both heads first (8 live p tiles)
                    p_tiles = {}
                    for hl in range(2):
                        for kc2 in range(NKC // 2):
                            sc = ps_sc.tile([P, 2, SQ], F32, tag="sc")
                            for j in range(2):
                                kc = 2 * kc2 + j
                                nc.tensor.matmul(
                                    sc[:, j, :],
                                    lhsT=khT2[hl * DK:(hl + 1) * DK,
                                              kc * P:(kc + 1) * P],
                                    rhs=qhT2[hl * DK:(hl + 1) * DK, :],
                                    start=True, stop=True)
                            p_sb = pwork.tile([P, 2, SQ], BF16, tag="p_sb")
                            nc.scalar.activation(
                                out=p_sb, in_=sc, func=AF.Exp, bias=zero1)
                            nc.vector.tensor_mul(
                                p_sb, p_sb, maskT[:, 2 * kc2:2 * kc2 + 2, :])
                            p_tiles[(hl, kc2)] = p_sb
                    # PV in [q, dv] orientation; col 64 = softmax row-sum
                    for hl in range(2):
                        h = 2 * pair + hl
                        hq = ps_hd.tile([P, NQT, DV + 1], F32, tag="hq")
                        for qt in range(NQT):
                            for kc2 in range(NKC // 2):
                                for j in range(2):
                                    kc = 2 * kc2 + j
                                    nc.tensor.matmul(
                                        hq[:, qt, :],
                                        lhsT=p_tiles[(hl, kc2)][
                                            :, j, qt * P:(qt + 1) * P],
                                        rhs=vh_sb[:, kc, h, 0:DV + 1],
                                        start=(kc == 0), stop=(kc == NKC - 1))
                        # normalize: cols 0:64 divided by col 64 (the rowsum)
                        rq = work.tile([P, NQT], F32, tag="rq")
                        nc.vector.reciprocal(out=rq, in_=hq[:, :, DV:DV + 1])
                        for qt in range(NQT):
                            nc.vector.tensor_scalar_mul(
                                heads_sb[:, qt, hl * DV:(hl + 1) * DV],
                                hq[:, qt, 0:DV], rq[:, qt:qt + 1])
                    # bounce through DRAM + XBAR transpose into concatT
                    cb = cbounce.tile([SQ, 2 * DV], BF16, tag="cb")
                    nc.sync.dma_start(
                        out=cb.rearrange("(qt p) hv -> p qt hv", p=P),
                        in_=heads_sb)
                    nc.sync.dma_start_transpose(
                        concatT[:, pair, :], cb[:])

            # --------------------------------------------------------------
            # Phase D: fc (out = concat @ fc_w.T), residual, LayerNorm.
            # --------------------------------------------------------------
            with tc.tile_pool(name="ps_d", bufs=4, space="PSUM") as ps_d:
                for st in range(NQT):
                    o_sb = work.tile([P, D], F32, tag="o_sb")
                    for half in range(2):
                        fps = ps_d.tile([P, 512], F32, tag="fps")
                        for ic in range(NDC):
                            nc.tensor.matmul(
                                fps,
                                lhsT=concatT[:, ic, st * P:(st + 1) * P],
                                rhs=fcT[:, ic, half * 512:(half + 1) * 512],
                                start=(ic == 0), stop=(ic == NDC - 1))
                        nc.vector.tensor_add(
                            o_sb[:, half * 512:(half + 1) * 512], fps,
                            qres[:, st, half * 512:(half + 1) * 512])
                    # LayerNorm over the 1024 free elements
                    stats = work.tile([P, 2, 6], F32, tag="stats")
                    for sg in range(2):
                        nc.vector.bn_stats(
                            out=stats[:, sg, :],
                            in_=o_sb[:, sg * 512:(sg + 1) * 512])
                    mv = work.tile([P, 2], F32, tag="mv")
                    nc.vector.bn_aggr(out=mv, in_=stats)
                    std = work.tile([P, 1], F32, tag="std")
                    nc.scalar.activation(
                        out=std, in_=mv[:, 1:2], func=AF.Sqrt, bias=eps1)
                    rstd = work.tile([P, 1], F32, tag="rstd")
                    nc.vector.reciprocal(out=rstd, in_=std)
                    nc.vector.tensor_scalar(
                        out=o_sb, in0=o_sb, scalar1=mv[:, 0:1], scalar2=rstd,
                        op0=mybir.AluOpType.subtract, op1=mybir.AluOpType.mult)
                    nc.vector.tensor_mul(o_sb, o_sb, gb[:, 0, :])
                    nc.vector.tensor_add(o_sb, o_sb, gb[:, 1, :])
                    nc.scalar.dma_start(
                        out=o_d[st * P:(st + 1) * P, :], in_=o_sb)

        for _rep in range(reps):
            _one_rep()

    nc.compile()
    return nc


_CACHE = {}


def _get_program():
    if "nc" not in _CACHE:
        _CACHE["nc"] = build_program()
    return _CACHE["nc"]


def make_in_maps(q, k, v, mask, wq, wk, wv, fc_w, ln_g, ln_b):
    q = np.asarray(q, dtype=np.float32)
    k = np.asarray(k, dtype=np.float32)
    v = np.asarray(v, dtype=np.float32)
    mask = np.asarray(mask, dtype=np.int32)
    wq = np.asarray(wq, dtype=np.float32) * 0.125  # fold in 1/sqrt(DK)
    wk = np.asarray(wk, dtype=np.float32)
    wv = np.asarray(wv, dtype=np.float32)
    fc_w = np.asarray(fc_w, dtype=np.float32)

    # wkq_l[pair, p, dc, 0:128]=wk two heads, [.., 128:256]=wq two heads
    def _pairs(w):  # [H, D, DK] -> [NPAIR, P, NDC, 2*DK]
        # [pair, 2, (dc p), dk] -> [pair, p, dc, 2*dk]
        a = w.reshape(NPAIR, 2, NDC, P, DK)
        return a.transpose(0, 3, 2, 1, 4).reshape(NPAIR, P, NDC, 2 * DK)

    wkq_l = np.concatenate([_pairs(wk), _pairs(wq)], axis=-1).astype(BF)
    # wv_l[p, dc, h*DV+v] = wv[h, dc*P+p, v]
    wv_l = np.ascontiguousarray(
        wv.transpose(1, 0, 2).reshape(NDC, P, H * DV).transpose(1, 0, 2)
    ).astype(BF)
    # fcT_l[p, ic, o] = fc_w[o, ic*P+p]
    fcT_l = np.ascontiguousarray(
        fc_w.T.reshape(NDC, P, D).transpose(1, 0, 2)).astype(BF)

    shared = {
        "wkq_l": np.ascontiguousarray(wkq_l),
        "wv_l": np.ascontiguousarray(wv_l),
        "fcT_l": np.ascontiguousarray(fcT_l),
        "ln_g": np.ascontiguousarray(np.asarray(ln_g, dtype=np.float32)),
        "ln_b": np.ascontiguousarray(np.asarray(ln_b, dtype=np.float32)),
    }
    q_bf = q.astype(BF)
    k_bf = k.astype(BF)
    v_bf = v.astype(BF)
    maskT_bf = mask.transpose(0, 2, 1).astype(BF)  # [B, S(k), S(q)]
    in_maps = []
    for c in range(N_CORES):
        b, half = c // 2, c % 2
        sl = slice(half * SQ, (half + 1) * SQ)
        in_maps.append({
            "q_sh": np.ascontiguousarray(q_bf[b, sl, :]),
            "k_full": np.ascontiguousarray(k_bf[b]),
            "v_full": np.ascontiguousarray(v_bf[b]),
            "maskT_sh": np.ascontiguousarray(maskT_bf[b, :, sl]),
            **shared,
        })
    return in_maps


def run(inputs: dict, trace: bool = False):
    nc = _get_program()
    in_maps = make_in_maps(**inputs)
    res = run_bass_kernel_spmd(
        nc, in_maps, core_ids=list(range(N_CORES)), trace=trace)
    out = np.empty((B, S, D), dtype=np.float32)
    for c in range(N_CORES):
        b, half = c // 2, c % 2
        out[b, half * SQ:(half + 1) * SQ, :] = res.results[c]["out_sh"]
    return out, res


def kernel(q, k, v, mask, wq, wk, wv, fc_w, ln_g, ln_b):
    out, _ = run(dict(q=q, k=k, v=v, mask=mask, wq=wq, wk=wk, wv=wv,
                      fc_w=fc_w, ln_g=ln_g, ln_b=ln_b))
    return out


# revision 18
# speedup vs baseline: 1.1283x; 1.1283x over previous
"""Trainium2 Bass kernel for a fused MultiHeadAttention block.

Reference computation (B=4, S=1024, D=1024, H=16, DK=DV=64):
    qh = einsum('bqd,hdk->bhqk', q, wq); kh, vh likewise
    attn = softmax(mask_fill(qh/sqrt(DK) @ kh^T))
    out  = LayerNorm(concat_heads(attn @ vh) @ fc_w.T + q) * ln_g + ln_b

Sharding: 8 shards = (batch b, seq half).  Each core owns 512 query rows of
one batch; K/V projections for that batch are computed redundantly by the
core pair.  Zero collectives.

Per-core strategy (bf16 everywhere, zero PE transposes, minimal PE
moving-columns — the HW is bound by PE column streaming at ~0.8ns/col):
  - all inputs/weights are cast to bf16 on the host; weights are also
    pre-arranged into their SBUF layouts (partition-major, >=4KB DMA rows)
    and wq is pre-scaled by 1/temperature; mask is pre-transposed to
    [k, q] bf16.
  - q/k/v are transposed on the way in by the DMA engines' XBAR
    (dma_start_transpose, 2-byte dtypes) — the PE does only real matmuls.
  - scores are computed TRANSPOSED [k_part, q_free], so softmax needs no
    max pass (|scores| <~ 6 sigma, exp cannot overflow) and no transpose of
    the 8.4M-element score tensor.
  - masking is p = exp(scores) * mask  (bitwise-identical to the reference's
    -1e9 masked-fill + softmax: both give exactly 0 weight).
  - PV runs with p as the STATIONARY operand and vh as the moving one, so
    its output lands in [q_part, dv] orientation with only 65 moving
    columns per matmul (64 dv + a ones-column that yields the softmax
    row-sum for free) — half the columns of the [dv, q] orientation.
    Normalization is then a per-partition reciprocal+tensor_scalar (no
    partition broadcast needed).
  - per-pair head outputs bounce through DRAM and are XBAR-transposed back
    into the [hv_part, q] layout the fc matmul needs; both transfers
    overlap attention of later pairs.
  - rel err vs the fp32 reference: ~1.8e-3 (bf16-dominated).
"""

import os
import sys

import numpy as np

for _p in ("/opt/trn_rl_repo",):
    if _p not in sys.path and os.path.isdir(_p):
        sys.path.insert(0, _p)

from contextlib import ExitStack

import ml_dtypes

import concourse.bass as bass
import concourse.tile as tile
from concourse import bacc, mybir
from concourse.bass_utils import run_bass_kernel_spmd

F32 = mybir.dt.float32
BF16 = mybir.dt.bfloat16
AF = mybir.ActivationFunctionType

B, S, D = 4, 1024, 1024
H, DK, DV = 16, 64, 64
SQ = S // 2          # query rows per core
P = 128
NDC = D // P         # 8 contraction chunks over D
NKC = S // P         # 8 key chunks
NQT = SQ // P        # 4 query subtiles
NPAIR = H // 2       # 8 head pairs
LN_EPS = 1e-6
N_CORES = 8
BF = ml_dtypes.bfloat16


def build_program(reps: int = 1):
    nc = bacc.Bacc("TRN2", target_bir_lowering=False, debug=False)

    q_d = nc.dram_tensor("q_sh", [SQ, D], BF16, kind="ExternalInput")
    k_d = nc.dram_tensor("k_full", [S, D], BF16, kind="ExternalInput")
    v_d = nc.dram_tensor("v_full", [S, D], BF16, kind="ExternalInput")
    mT_d = nc.dram_tensor("maskT_sh", [S, SQ], BF16, kind="ExternalInput")
    wkq_d = nc.dram_tensor("wkq_l", [NPAIR, P, NDC, 4 * DK], BF16,
                           kind="ExternalInput")
    wv_d = nc.dram_tensor("wv_l", [P, NDC, H * DV], BF16, kind="ExternalInput")
    fc_d = nc.dram_tensor("fcT_l", [P, NDC, D], BF16, kind="ExternalInput")
    g_d = nc.dram_tensor("ln_g", [D], F32, kind="ExternalInput")
    b_d = nc.dram_tensor("ln_b", [D], F32, kind="ExternalInput")
    o_d = nc.dram_tensor("out_sh", [SQ, D], F32, kind="ExternalOutput")

    with tile.TileContext(nc) as tc, ExitStack() as ctx:
        singles = ctx.enter_context(tc.tile_pool(name="singles", bufs=1))
        bigs = ctx.enter_context(tc.tile_pool(name="bigs", bufs=1))
        work = ctx.enter_context(tc.tile_pool(name="work", bufs=2))
        pwork = ctx.enter_context(tc.tile_pool(name="pwork", bufs=8))

        zero1 = singles.tile([P, 1], F32, tag="zero1")
        nc.vector.memset(zero1, 0.0)
        eps1 = singles.tile([P, 1], F32, tag="eps1")
        nc.vector.memset(eps1, LN_EPS)

        def _one_rep():
            # --------------------------------------------------------------
            # Loads.  SP queue: v/k/q DMA-transposes + per-pair wkq.
            # Act queue: wv, mask, fc, residual, ln params.
            # --------------------------------------------------------------
            vT = bigs.tile([P, NDC, S], BF16, tag="vT")
            kT = bigs.tile([P, NDC, S], BF16, tag="kT")
            qT = bigs.tile([P, NDC, SQ], BF16, tag="qT")
            maskT = bigs.tile([P, NKC, SQ], BF16, tag="maskT")
            wv_sb = bigs.tile([P, NDC, H * DV], BF16, tag="wv")
            fcT = bigs.tile([P, NDC, D], BF16, tag="fcT")
            qres = bigs.tile([P, NQT, D], BF16, tag="qres")
            gb = bigs.tile([P, 2, D], F32, tag="gb")

            nc.scalar.dma_start(out=wv_sb[:, :, 0:512], in_=wv_d[:, :, 0:512])
            nc.scalar.dma_start(out=wv_sb[:, :, 512:], in_=wv_d[:, :, 512:])
            for kc in range(NKC):
                nc.sync.dma_start_transpose(
                    vT[:, :, kc * P:(kc + 1) * P],
                    v_d[kc * P:(kc + 1) * P, :])
            for half in range(2):
                nc.sync.dma_start_transpose(
                    kT[:, :, half * SQ:(half + 1) * SQ],
                    k_d[half * SQ:(half + 1) * SQ, :])
            nc.sync.dma_start_transpose(qT, q_d.ap())
            nc.scalar.dma_start(
                out=maskT, in_=mT_d.ap().rearrange("(kc p) q -> p kc q", p=P))
            nc.scalar.dma_start(out=fcT, in_=fc_d.ap())
            nc.scalar.dma_start(
                out=qres, in_=q_d.ap().rearrange("(t p) d -> p t d", p=P))
            nc.scalar.dma_start(
                out=gb[:, 0, :], in_=g_d.ap().unsqueeze(0).to_broadcast([P, D]))
            nc.scalar.dma_start(
                out=gb[:, 1, :], in_=b_d.ap().unsqueeze(0).to_broadcast([P, D]))

            # --------------------------------------------------------------
            # Phase B: vh for all heads.  vh_sb[kp, kc, h, 0:64] = vh,
            # col 64 = 1, cols 65:128 = 0 (free softmax row-sums).
            # --------------------------------------------------------------
            vh_sb = bigs.tile([P, NKC, H, 2 * DV], BF16, tag="vh")
            nc.vector.memset(vh_sb[:, :, :, DV:].bitcast(mybir.dt.uint32), 0)
            nc.vector.memset(
                vh_sb[:, :, :, DV:DV + 2].bitcast(mybir.dt.uint32),
                0x00003F80)  # bf16 pair [1.0, 0.0] little-endian

            with tc.tile_pool(name="ps_b", bufs=4, space="PSUM") as ps_b:
                for kc in range(NKC):
                    for half in range(2):
                        vps = ps_b.tile([P, 512], F32, tag="vps")
                        for dj in range(NDC):
                            nc.tensor.matmul(
                                vps, lhsT=vT[:, dj, kc * P:(kc + 1) * P],
                                rhs=wv_sb[:, dj, half * 512:(half + 1) * 512],
                                start=(dj == 0), stop=(dj == NDC - 1))
                        nc.scalar.copy(
                            out=vh_sb[:, kc, half * 8:(half + 1) * 8, 0:DV],
                            in_=vps.rearrange("p (h v) -> p h v", v=DV))

            # --------------------------------------------------------------
            # Phase C: per head-pair projections + attention.
            # concatT[ip, pair, q] rows: head 2*pair in 0:64, 2*pair+1 in
            # 64:128
            # --------------------------------------------------------------
            concatT = bigs.tile([P, NPAIR, SQ], BF16, tag="concatT")

            with (
                tc.tile_pool(name="ps_kq", bufs=2, space="PSUM") as ps_kq,
                tc.tile_pool(name="ps_sc", bufs=2, space="PSUM") as ps_sc,
                tc.tile_pool(name="ps_hd", bufs=2, space="PSUM") as ps_hd,
                tc.tile_pool(name="cbounce", bufs=2, space="DRAM") as cbounce,
            ):
                for pair in range(NPAIR):
                    wkq = work.tile([P, NDC, 4 * DK], BF16, tag="wkq")
                    nc.sync.dma_start(out=wkq, in_=wkq_d[pair])

                    # khT2: [128 (2h x dk), 1024 kcol]
                    khT2 = work.tile([P, S], BF16, tag="khT2")
                    for half in range(2):
                        khps = ps_kq.tile([P, 512], F32, tag="kqps")
                        for dj in range(NDC):
                            nc.tensor.matmul(
                                khps, lhsT=wkq[:, dj, 0:2 * DK],
                                rhs=kT[:, dj, half * 512:(half + 1) * 512],
                                start=(dj == 0), stop=(dj == NDC - 1))
                        nc.vector.tensor_copy(
                            out=khT2[:, half * 512:(half + 1) * 512], in_=khps)
                    # qhT2: [128 (2h x dk), 512 q]  (wq pre-scaled by 1/temp)
                    qhT2 = work.tile([P, SQ], BF16, tag="qhT2")
                    qhps = ps_kq.tile([P, SQ], F32, tag="kqps")
                    for dj in range(NDC):
                        nc.tensor.matmul(
                            qhps, lhsT=wkq[:, dj, 2 * DK:4 * DK],
                            rhs=qT[:, dj, :],
                            start=(dj == 0), stop=(dj == NDC - 1))
                    nc.vector.tensor_copy(out=qhT2, in_=qhps)

                    # heads_sb[q_part, qt, hl*64+dv] -- [q, dv] orientation
                    heads_sb = work.tile([P, NQT, 2 * DV], BF16, tag="heads")
                    # All probabilities for both heads first (8 live p tiles)
                    p_tiles = {}
                    for hl in range(2):
                        for kc2 in range(NKC // 2):
                            sc = ps_sc.tile([P, 2, SQ], F32, tag="sc")
                            for j in range(2):
                                kc = 2 * kc2 + j
                                nc.tensor.matmul(
                                    sc[:, j, :],
                                    lhsT=khT2[hl * DK:(hl + 1) * DK,
                                              kc * P:(kc + 1) * P],
                                    rhs=qhT2[hl * DK:(hl + 1) * DK, :],
                                    start=True, stop=True)
                            p_sb = pwork.tile([P, 2, SQ], BF16, tag="p_sb")
                            nc.scalar.activation(
                                out=p_sb, in_=sc, func=AF.Exp, bias=zero1)
                            nc.vector.tensor_mul(
                                p_sb, p_sb, maskT[:, 2 * kc2:2 * kc2 + 2, :])
                            p_tiles[(hl, kc2)] = p_sb
                    # PV in [q, dv] orientation; col 64 = softmax row-sum
                    for hl in range(2):
                        h = 2 * pair + hl
                        hq = ps_hd.tile([P, NQT, DV + 1], F32, tag="hq")
                        for qt in range(NQT):
                            for kc2 in range(NKC // 2):
                                for j in range(2):
                                    kc = 2 * kc2 + j
                                    nc.tensor.matmul(
                                        hq[:, qt, :],
                                        lhsT=p_tiles[(hl, kc2)][
                                            :, j, qt * P:(qt + 1) * P],
                                        rhs=vh_sb[:, kc, h, 0:DV + 1],
                                        start=(kc == 0), stop=(kc == NKC - 1))
                        # normalize: cols 0:64 divided by col 64 (the rowsum)
                        rq = work.tile([P, NQT], F32, tag="rq")
                        nc.vector.reciprocal(out=rq, in_=hq[:, :, DV:DV + 1])
                        for qt in range(NQT):
                            nc.vector.tensor_scalar_mul(
                                heads_sb[:, qt, hl * DV:(hl + 1) * DV],
                                hq[:, qt, 0:DV], rq[:, qt:qt + 1])
                    # bounce through DRAM + XBAR transpose into concatT
                    cb = cbounce.tile([SQ, 2 * DV], BF16, tag="cb")
                    nc.sync.dma_start(
                        out=cb.rearrange("(qt p) hv -> p qt hv", p=P),
                        in_=heads_sb)
                    nc.sync.dma_start_transpose(
                        concatT[:, pair, :], cb[:])

            # --------------------------------------------------------------
            # Phase D: fc (out = concat @ fc_w.T), residual, LayerNorm.
            # --------------------------------------------------------------
            with tc.tile_pool(name="ps_d", bufs=4, space="PSUM") as ps_d:
                for st in range(NQT):
                    o_sb = work.tile([P, D], F32, tag="o_sb")
                    for half in range(2):
                        fps = ps_d.tile([P, 512], F32, tag="fps")
                        for ic in range(NDC):
                            nc.tensor.matmul(
                                fps,
                                lhsT=concatT[:, ic, st * P:(st + 1) * P],
                                rhs=fcT[:, ic, half * 512:(half + 1) * 512],
                                start=(ic == 0), stop=(ic == NDC - 1))
                        nc.vector.tensor_add(
                            o_sb[:, half * 512:(half + 1) * 512], fps,
                            qres[:, st, half * 512:(half + 1) * 512])
                    # LayerNorm over the 1024 free elements
                    stats = work.tile([P, 2, 6], F32, tag="stats")
                    for sg in range(2):
                        nc.vector.bn_stats(
                            out=stats[:, sg, :],
                            in_=o_sb[:, sg * 512:(sg + 1) * 512])
                    mv = work.tile([P, 2], F32, tag="mv")
                    nc.vector.bn_aggr(out=mv, in_=stats)
                    std = work.tile([P, 1], F32, tag="std")
                    nc.scalar.activation(
                        out=std, in_=mv[:, 1:2], func=AF.Sqrt, bias=eps1)
                    rstd = work.tile([P, 1], F32, tag="rstd")
                    nc.vector.reciprocal(out=rstd, in_=std)
                    nc.vector.tensor_scalar(
                        out=o_sb, in0=o_sb, scalar1=mv[:, 0:1], scalar2=rstd,
                        op0=mybir.AluOpType.subtract, op1=mybir.AluOpType.mult)
                    nc.vector.tensor_mul(o_sb, o_sb, gb[:, 0, :])
                    nc.vector.tensor_add(o_sb, o_sb, gb[:, 1, :])
                    nc.sync.dma_start(out=o_d[st * P:(st + 1) * P, :], in_=o_sb)

        for _rep in range(reps):
            _one_rep()

    nc.compile()
    return nc


_CACHE = {}


def _get_program():
    if "nc" not in _CACHE:
        _CACHE["nc"] = build_program()
    return _CACHE["nc"]


def make_in_maps(q, k, v, mask, wq, wk, wv, fc_w, ln_g, ln_b):
    q = np.asarray(q, dtype=np.float32)
    k = np.asarray(k, dtype=np.float32)
    v = np.asarray(v, dtype=np.float32)
    mask = np.asarray(mask, dtype=np.int32)
    wq = np.asarray(wq, dtype=np.float32) * 0.125  # fold in 1/sqrt(DK)
    wk = np.asarray(wk, dtype=np.float32)
    wv = np.asarray(wv, dtype=np.float32)
    fc_w = np.asarray(fc_w, dtype=np.float32)

    # wkq_l[pair, p, dc, 0:128]=wk two heads, [.., 128:256]=wq two heads
    def _pairs(w):  # [H, D, DK] -> [NPAIR, P, NDC, 2*DK]
        # [pair, 2, (dc p), dk] -> [pair, p, dc, 2*dk]
        a = w.reshape(NPAIR, 2, NDC, P, DK)
        return a.transpose(0, 3, 2, 1, 4).reshape(NPAIR, P, NDC, 2 * DK)

    wkq_l = np.concatenate([_pairs(wk), _pairs(wq)], axis=-1).astype(BF)
    # wv_l[p, dc, h*DV+v] = wv[h, dc*P+p, v]
    wv_l = np.ascontiguousarray(
        wv.transpose(1, 0, 2).reshape(NDC, P, H * DV).transpose(1, 0, 2)
    ).astype(BF)
    # fcT_l[p, ic, o] = fc_w[o, ic*P+p]
    fcT_l = np.ascontiguousarray(
        fc_w.T.reshape(NDC, P, D).transpose(1, 0, 2)).astype(BF)

    shared = {
        "wkq_l": np.ascontiguousarray(wkq_l),
        "wv_l": np.ascontiguousarray(wv_l),
        "fcT_l": np.ascontiguousarray(fcT_l),
        "ln_g": np.ascontiguousarray(np.asarray(ln_g, dtype=np.float32)),
        "ln_b": np.ascontiguousarray(np.asarray(ln_b, dtype=np.float32)),
    }
    q_bf = q.astype(BF)
    k_bf = k.astype(BF)
    v_bf = v.astype(BF)
    maskT_bf = mask.transpose(0, 2, 1).astype(BF)  # [B, S(k), S(q)]
    in_maps = []
    for c in range(N_CORES):
        b, half = c // 2, c % 2
        sl = slice(half * SQ, (half + 1) * SQ)
        in_maps.append({
            "q_sh": np.ascontiguousarray(q_bf[b, sl, :]),
            "k_full": np.ascontiguousarray(k_bf[b]),
            "v_full": np.ascontiguousarray(v_bf[b]),
            "maskT_sh": np.ascontiguousarray(maskT_bf[b, :, sl]),
            **shared,
        })
    return in_maps


def run(inputs: dict, trace: bool = False):
    nc = _get_program()
    in_maps = make_in_maps(**inputs)
    res = run_bass_kernel_spmd(
        nc, in_maps, core_ids=list(range(N_CORES)), trace=trace)
    out = np.empty((B, S, D), dtype=np.float32)
    for c in range(N_CORES):
        b, half = c // 2, c % 2
        out[b, half * SQ:(half + 1) * SQ, :] = res.results[c]["out_sh"]
    return out, res


def kernel(q, k, v, mask, wq, wk, wv, fc_w, ln_g, ln_b):
    out, _ = run(dict(q=q, k=k, v=v, mask=mask, wq=wq, wk=wk, wv=wv,
                      fc_w=fc_w, ln_g=ln_g, ln_b=ln_b))
    return out


# revision 20
# speedup vs baseline: 1.2028x; 1.0660x over previous
"""Trainium2 Bass kernel for a fused MultiHeadAttention block.

Reference computation (B=4, S=1024, D=1024, H=16, DK=DV=64):
    qh = einsum('bqd,hdk->bhqk', q, wq); kh, vh likewise
    attn = softmax(mask_fill(qh/sqrt(DK) @ kh^T))
    out  = LayerNorm(concat_heads(attn @ vh) @ fc_w.T + q) * ln_g + ln_b

Sharding: 8 shards = (batch b, seq half).  Each core owns 512 query rows of
one batch; K/V projections for that batch are computed redundantly by the
core pair.  Zero collectives.

Per-core strategy (bf16 everywhere, zero PE transposes, minimal PE
moving-columns — the HW is bound by PE column streaming at ~0.8ns/col):
  - all inputs/weights are cast to bf16 on the host; weights are also
    pre-arranged into their SBUF layouts (partition-major, >=4KB DMA rows)
    and wq is pre-scaled by 1/temperature; mask is pre-transposed to
    [k, q] bf16.
  - q/k/v are transposed on the way in by the DMA engines' XBAR
    (dma_start_transpose, 2-byte dtypes) — the PE does only real matmuls.
  - scores are computed TRANSPOSED [k_part, q_free], so softmax needs no
    max pass (|scores| <~ 6 sigma, exp cannot overflow) and no transpose of
    the 8.4M-element score tensor.
  - masking is p = exp(scores) * mask  (bitwise-identical to the reference's
    -1e9 masked-fill + softmax: both give exactly 0 weight).
  - PV runs with p as the STATIONARY operand and vh as the moving one, so
    its output lands in [q_part, dv] orientation with only 65 moving
    columns per matmul (64 dv + a ones-column that yields the softmax
    row-sum for free) — half the columns of the [dv, q] orientation.
    Normalization is then a per-partition reciprocal+tensor_scalar (no
    partition broadcast needed).
  - per-pair head outputs bounce through DRAM and are XBAR-transposed back
    into the [hv_part, q] layout the fc matmul needs; both transfers
    overlap attention of later pairs.
  - vh_sb stores only 66 columns per head (64 values + ones + pad) and is
    double-buffered; with the output DMAs on the Act queue and all
    PE-critical loads on the SP queue, each iteration's input
    DMA-transposes prefetch during the previous iteration's fc/LayerNorm
    phase.
  - rel err vs the fp32 reference: ~1.8e-3 (bf16-dominated).
"""

import os
import sys

import numpy as np

for _p in ("/opt/trn_rl_repo",):
    if _p not in sys.path and os.path.isdir(_p):
        sys.path.insert(0, _p)

from contextlib import ExitStack

import ml_dtypes

import concourse.bass as bass
import concourse.tile as tile
from concourse import bacc, mybir
from concourse.bass_utils import run_bass_kernel_spmd

F32 = mybir.dt.float32
BF16 = mybir.dt.bfloat16
AF = mybir.ActivationFunctionType

B, S, D = 4, 1024, 1024
H, DK, DV = 16, 64, 64
SQ = S // 2          # query rows per core
P = 128
NDC = D // P         # 8 contraction chunks over D
NKC = S // P         # 8 key chunks
NQT = SQ // P        # 4 query subtiles
NPAIR = H // 2       # 8 head pairs
LN_EPS = 1e-6
N_CORES = 8
BF = ml_dtypes.bfloat16


def build_program(reps: int = 1):
    nc = bacc.Bacc("TRN2", target_bir_lowering=False, debug=False)

    q_d = nc.dram_tensor("q_sh", [SQ, D], BF16, kind="ExternalInput")
    k_d = nc.dram_tensor("k_full", [S, D], BF16, kind="ExternalInput")
    v_d = nc.dram_tensor("v_full", [S, D], BF16, kind="ExternalInput")
    mT_d = nc.dram_tensor("maskT_sh", [S, SQ], BF16, kind="ExternalInput")
    wkq_d = nc.dram_tensor("wkq_l", [NPAIR, P, NDC, 4 * DK], BF16,
                           kind="ExternalInput")
    wv_d = nc.dram_tensor("wv_l", [P, NDC, H * DV], BF16, kind="ExternalInput")
    fc_d = nc.dram_tensor("fcT_l", [P, NDC, D], BF16, kind="ExternalInput")
    g_d = nc.dram_tensor("ln_g", [D], F32, kind="ExternalInput")
    b_d = nc.dram_tensor("ln_b", [D], F32, kind="ExternalInput")
    o_d = nc.dram_tensor("out_sh", [SQ, D], F32, kind="ExternalOutput")

    with tile.TileContext(nc) as tc, ExitStack() as ctx:
        singles = ctx.enter_context(tc.tile_pool(name="singles", bufs=1))
        bigs = ctx.enter_context(tc.tile_pool(name="bigs", bufs=1))
        vhpool = ctx.enter_context(tc.tile_pool(name="vhpool", bufs=2))
        work = ctx.enter_context(tc.tile_pool(name="work", bufs=2))
        pwork = ctx.enter_context(tc.tile_pool(name="pwork", bufs=8))

        zero1 = singles.tile([P, 1], F32, tag="zero1")
        nc.vector.memset(zero1, 0.0)
        eps1 = singles.tile([P, 1], F32, tag="eps1")
        nc.vector.memset(eps1, LN_EPS)

        def _one_rep():
            # --------------------------------------------------------------
            # Loads.  SP queue: v/k/q DMA-transposes + per-pair wkq.
            # Act queue: wv, mask, fc, residual, ln params.
            # --------------------------------------------------------------
            vT = bigs.tile([P, NDC, S], BF16, tag="vT")
            kT = bigs.tile([P, NDC, S], BF16, tag="kT")
            qT = bigs.tile([P, NDC, SQ], BF16, tag="qT")
            maskT = bigs.tile([P, NKC, SQ], BF16, tag="maskT")
            wv_sb = bigs.tile([P, NDC, H * DV], BF16, tag="wv")
            fcT = bigs.tile([P, NDC, D], BF16, tag="fcT")
            qres = bigs.tile([P, NQT, D], BF16, tag="qres")
            gb = bigs.tile([P, 2, D], F32, tag="gb")

            # SP queue carries everything the next rep's PE start needs, so
            # it prefetches during the previous rep's phase D (the out DMAs
            # live on the Act queue and don't block it).
            nc.sync.dma_start_transpose(
                vT[:, :, 0:P], v_d[0:P, :])
            nc.sync.dma_start(out=wv_sb[:, :, 0:512], in_=wv_d[:, :, 0:512])
            nc.sync.dma_start(out=wv_sb[:, :, 512:], in_=wv_d[:, :, 512:])
            for kc in range(1, NKC):
                nc.sync.dma_start_transpose(
                    vT[:, :, kc * P:(kc + 1) * P],
                    v_d[kc * P:(kc + 1) * P, :])
            for half in range(2):
                nc.sync.dma_start_transpose(
                    kT[:, :, half * SQ:(half + 1) * SQ],
                    k_d[half * SQ:(half + 1) * SQ, :])
            nc.sync.dma_start_transpose(qT, q_d.ap())
            nc.scalar.dma_start(
                out=maskT, in_=mT_d.ap().rearrange("(kc p) q -> p kc q", p=P))
            nc.scalar.dma_start(out=fcT, in_=fc_d.ap())
            nc.scalar.dma_start(
                out=qres, in_=q_d.ap().rearrange("(t p) d -> p t d", p=P))
            nc.scalar.dma_start(
                out=gb[:, 0, :], in_=g_d.ap().unsqueeze(0).to_broadcast([P, D]))
            nc.scalar.dma_start(
                out=gb[:, 1, :], in_=b_d.ap().unsqueeze(0).to_broadcast([P, D]))

            # --------------------------------------------------------------
            # Phase B: vh for all heads.  vh_sb[kp, kc, h, 0:64] = vh,
            # col 64 = 1 (free softmax row-sums), col 65 = alignment pad.
            # Double-buffered so the next rep's phase B doesn't wait for
            # this rep's last PV read.
            # --------------------------------------------------------------
            vh_sb = vhpool.tile([P, NKC, H, DV + 2], BF16, tag="vh")
            nc.vector.memset(
                vh_sb[:, :, :, DV:DV + 2].bitcast(mybir.dt.uint32),
                0x00003F80)  # bf16 pair [1.0, 0.0] little-endian

            with tc.tile_pool(name="ps_b", bufs=4, space="PSUM") as ps_b:
                for kc in range(NKC):
                    for half in range(2):
                        vps = ps_b.tile([P, 512], F32, tag="vps")
                        for dj in range(NDC):
                            nc.tensor.matmul(
                                vps, lhsT=vT[:, dj, kc * P:(kc + 1) * P],
                                rhs=wv_sb[:, dj, half * 512:(half + 1) * 512],
                                start=(dj == 0), stop=(dj == NDC - 1))
                        nc.scalar.copy(
                            out=vh_sb[:, kc, half * 8:(half + 1) * 8, 0:DV],
                            in_=vps.rearrange("p (h v) -> p h v", v=DV))

            # --------------------------------------------------------------
            # Phase C: per head-pair projections + attention.
            # concatT[ip, pair, q] rows: head 2*pair in 0:64, 2*pair+1 in
            # 64:128
            # --------------------------------------------------------------
            concatT = bigs.tile([P, NPAIR, SQ], BF16, tag="concatT")

            with (
                tc.tile_pool(name="ps_kq", bufs=2, space="PSUM") as ps_kq,
                tc.tile_pool(name="ps_sc", bufs=2, space="PSUM") as ps_sc,
                tc.tile_pool(name="ps_hd", bufs=2, space="PSUM") as ps_hd,
                tc.tile_pool(name="cbounce", bufs=2, space="DRAM") as cbounce,
            ):
                for pair in range(NPAIR):
                    wkq = work.tile([P, NDC, 4 * DK], BF16, tag="wkq")
                    nc.sync.dma_start(out=wkq, in_=wkq_d[pair])

                    # khT2: [128 (2h x dk), 1024 kcol]
                    khT2 = work.tile([P, S], BF16, tag="khT2")
                    for half in range(2):
                        khps = ps_kq.tile([P, 512], F32, tag="kqps")
                        for dj in range(NDC):
                            nc.tensor.matmul(
                                khps, lhsT=wkq[:, dj, 0:2 * DK],
                                rhs=kT[:, dj, half * 512:(half + 1) * 512],
                                start=(dj == 0), stop=(dj == NDC - 1))
                        nc.vector.tensor_copy(
                            out=khT2[:, half * 512:(half + 1) * 512], in_=khps)
                    # qhT2: [128 (2h x dk), 512 q]  (wq pre-scaled by 1/temp)
                    qhT2 = work.tile([P, SQ], BF16, tag="qhT2")
                    qhps = ps_kq.tile([P, SQ], F32, tag="kqps")
                    for dj in range(NDC):
                        nc.tensor.matmul(
                            qhps, lhsT=wkq[:, dj, 2 * DK:4 * DK],
                            rhs=qT[:, dj, :],
                            start=(dj == 0), stop=(dj == NDC - 1))
                    nc.vector.tensor_copy(out=qhT2, in_=qhps)

                    # heads_sb[q_part, qt, hl*64+dv] -- [q, dv] orientation
                    heads_sb = work.tile([P, NQT, 2 * DV], BF16, tag="heads")
                    # All probabilities for both heads first (8 live p tiles)
                    p_tiles = {}
                    for hl in range(2):
                        for kc2 in range(NKC // 2):
                            sc = ps_sc.tile([P, 2, SQ], F32, tag="sc")
                            for j in range(2):
                                kc = 2 * kc2 + j
                                nc.tensor.matmul(
                                    sc[:, j, :],
                                    lhsT=khT2[hl * DK:(hl + 1) * DK,
                                              kc * P:(kc + 1) * P],
                                    rhs=qhT2[hl * DK:(hl + 1) * DK, :],
                                    start=True, stop=True)
                            p_sb = pwork.tile([P, 2, SQ], BF16, tag="p_sb")
                            nc.scalar.activation(
                                out=p_sb, in_=sc, func=AF.Exp, bias=zero1)
                            nc.vector.tensor_mul(
                                p_sb, p_sb, maskT[:, 2 * kc2:2 * kc2 + 2, :])
                            p_tiles[(hl, kc2)] = p_sb
                    # PV in [q, dv] orientation; col 64 = softmax row-sum
                    for hl in range(2):
                        h = 2 * pair + hl
                        hq = ps_hd.tile([P, NQT, DV + 1], F32, tag="hq")
                        for qt in range(NQT):
                            for kc2 in range(NKC // 2):
                                for j in range(2):
                                    kc = 2 * kc2 + j
                                    nc.tensor.matmul(
                                        hq[:, qt, :],
                                        lhsT=p_tiles[(hl, kc2)][
                                            :, j, qt * P:(qt + 1) * P],
                                        rhs=vh_sb[:, kc, h, 0:DV + 1],
                                        start=(kc == 0), stop=(kc == NKC - 1))
                        # normalize: cols 0:64 divided by col 64 (the rowsum)
                        rq = work.tile([P, NQT], F32, tag="rq")
                        nc.vector.reciprocal(out=rq, in_=hq[:, :, DV:DV + 1])
                        for qt in range(NQT):
                            nc.vector.tensor_scalar_mul(
                                heads_sb[:, qt, hl * DV:(hl + 1) * DV],
                                hq[:, qt, 0:DV], rq[:, qt:qt + 1])
                    # bounce through DRAM + XBAR transpose into concatT
                    cb = cbounce.tile([SQ, 2 * DV], BF16, tag="cb")
                    nc.sync.dma_start(
                        out=cb.rearrange("(qt p) hv -> p qt hv", p=P),
                        in_=heads_sb)
                    nc.sync.dma_start_transpose(
                        concatT[:, pair, :], cb[:])

            # --------------------------------------------------------------
            # Phase D: fc (out = concat @ fc_w.T), residual, LayerNorm.
            # --------------------------------------------------------------
            with tc.tile_pool(name="ps_d", bufs=4, space="PSUM") as ps_d:
                for st in range(NQT):
                    o_sb = work.tile([P, D], F32, tag="o_sb")
                    for half in range(2):
                        fps = ps_d.tile([P, 512], F32, tag="fps")
                        for ic in range(NDC):
                            nc.tensor.matmul(
                                fps,
                                lhsT=concatT[:, ic, st * P:(st + 1) * P],
                                rhs=fcT[:, ic, half * 512:(half + 1) * 512],
                                start=(ic == 0), stop=(ic == NDC - 1))
                        nc.vector.tensor_add(
                            o_sb[:, half * 512:(half + 1) * 512], fps,
                            qres[:, st, half * 512:(half + 1) * 512])
                    # LayerNorm over the 1024 free elements
                    stats = work.tile([P, 2, 6], F32, tag="stats")
                    for sg in range(2):
                        nc.vector.bn_stats(
                            out=stats[:, sg, :],
                            in_=o_sb[:, sg * 512:(sg + 1) * 512])
                    mv = work.tile([P, 2], F32, tag="mv")
                    nc.vector.bn_aggr(out=mv, in_=stats)
                    std = work.tile([P, 1], F32, tag="std")
                    nc.scalar.activation(
                        out=std, in_=mv[:, 1:2], func=AF.Sqrt, bias=eps1)
                    rstd = work.tile([P, 1], F32, tag="rstd")
                    nc.vector.reciprocal(out=rstd, in_=std)
                    nc.vector.tensor_scalar(
                        out=o_sb, in0=o_sb, scalar1=mv[:, 0:1], scalar2=rstd,
                        op0=mybir.AluOpType.subtract, op1=mybir.AluOpType.mult)
                    nc.vector.tensor_mul(o_sb, o_sb, gb[:, 0, :])
                    nc.vector.tensor_add(o_sb, o_sb, gb[:, 1, :])
                    nc.scalar.dma_start(
                        out=o_d[st * P:(st + 1) * P, :], in_=o_sb)

        for _rep in range(reps):
            _one_rep()

    nc.compile()
    return nc


_CACHE = {}


def _get_program():
    if "nc" not in _CACHE:
        _CACHE["nc"] = build_program()
    return _CACHE["nc"]


def make_in_maps(q, k, v, mask, wq, wk, wv, fc_w, ln_g, ln_b):
    q = np.asarray(q, dtype=np.float32)
    k = np.asarray(k, dtype=np.float32)
    v = np.asarray(v, dtype=np.float32)
    mask = np.asarray(mask, dtype=np.int32)
    wq = np.asarray(wq, dtype=np.float32) * 0.125  # fold in 1/sqrt(DK)
    wk = np.asarray(wk, dtype=np.float32)
    wv = np.asarray(wv, dtype=np.float32)
    fc_w = np.asarray(fc_w, dtype=np.float32)

    # wkq_l[pair, p, dc, 0:128]=wk two heads, [.., 128:256]=wq two heads
    def _pairs(w):  # [H, D, DK] -> [NPAIR, P, NDC, 2*DK]
        # [pair, 2, (dc p), dk] -> [pair, p, dc, 2*dk]
        a = w.reshape(NPAIR, 2, NDC, P, DK)
        return a.transpose(0, 3, 2, 1, 4).reshape(NPAIR, P, NDC, 2 * DK)

    wkq_l = np.concatenate([_pairs(wk), _pairs(wq)], axis=-1).astype(BF)
    # wv_l[p, dc, h*DV+v] = wv[h, dc*P+p, v]
    wv_l = np.ascontiguousarray(
        wv.transpose(1, 0, 2).reshape(NDC, P, H * DV).transpose(1, 0, 2)
    ).astype(BF)
    # fcT_l[p, ic, o] = fc_w[o, ic*P+p]
    fcT_l = np.ascontiguousarray(
        fc_w.T.reshape(NDC, P, D).transpose(1, 0, 2)).astype(BF)

    shared = {
        "wkq_l": np.ascontiguousarray(wkq_l),
        "wv_l": np.ascontiguousarray(wv_l),
        "fcT_l": np.ascontiguousarray(fcT_l),
        "ln_g": np.ascontiguousarray(np.asarray(ln_g, dtype=np.float32)),
        "ln_b": np.ascontiguousarray(np.asarray(ln_b, dtype=np.float32)),
    }
    q_bf = q.astype(BF)
    k_bf = k.astype(BF)
    v_bf = v.astype(BF)
    maskT_bf = mask.transpose(0, 2, 1).astype(BF)  # [B, S(k), S(q)]
    in_maps = []
    for c in range(N_CORES):
        b, half = c // 2, c % 2
        sl = slice(half * SQ, (half + 1) * SQ)
        in_maps.append({
            "q_sh": np.ascontiguousarray(q_bf[b, sl, :]),
            "k_full": np.ascontiguousarray(k_bf[b]),
            "v_full": np.ascontiguousarray(v_bf[b]),
            "maskT_sh": np.ascontiguousarray(maskT_bf[b, :, sl]),
            **shared,
        })
    return in_maps


def run(inputs: dict, trace: bool = False):
    nc = _get_program()
    in_maps = make_in_maps(**inputs)
    res = run_bass_kernel_spmd(
        nc, in_maps, core_ids=list(range(N_CORES)), trace=trace)
    out = np.empty((B, S, D), dtype=np.float32)
    for c in range(N_CORES):
        b, half = c // 2, c % 2
        out[b, half * SQ:(half + 1) * SQ, :] = res.results[c]["out_sh"]
    return out, res


def kernel(q, k, v, mask, wq, wk, wv, fc_w, ln_g, ln_b):
    out, _ = run(dict(q=q, k=k, v=v, mask=mask, wq=wq, wk=wk, wv=wv,
                      fc_w=fc_w, ln_g=ln_g, ln_b=ln_b))
    return out
